# revision 24
# baseline (speedup 1.0000x reference)
"""Multi-head causal attention (B=2, L=2048, E=1024, H=16, D=64) on 8 NeuronCores.

Sharding: data-parallel over batch x tensor-parallel over heads.
  core c: batch b = c // 4, head group hg = c % 4 -> heads [4*hg, 4*hg+4).
Each core computes QKV projection for its 4 heads, causal softmax attention,
and a *partial* output projection (its heads' slice of Wout). The host sums
the 4 bf16 partial outputs per batch in f32 and adds the bias.

Device schedule (single emission stream; Tile framework inserts sems):
  - inputs DMA'd chunked: waT, x l-half 0, x l-half 1, woT.
  - serial head: QKV for pair 0 / l-half 0 + V(all heads, l-half 0).
  - attention sweeps in order p0s0, p1s0, p0s1, p1s1 (i-windows of 1024);
    the remaining QKV units (pair 1 lh0, V lh1, all lh1) drip into the
    p0s0/p1s0 streams as fillers so the PE stays busy during exp waits.
  - output projection accumulates BOTH head pairs in PSUM (no HBM
    read-modify-write); lb0 units drip into the sweep-1 streams, lb1 is
    a short tail. Output is written bf16; bias is added on the host.
  - softmax Z: ones-column in V gives Z on PSUM row 64; 1/Z via DVE
    reciprocal, broadcast across 64 partitions by GPSIMD
    partition_broadcast (replaces the PE broadcast matmul + bounce).

Device notes:
  - Matmul operands bf16 (fp32 PSUM accumulation); host pre-casts/transposes.
  - Attention runs in the S^T layout (scores[j, i]).
  - No max-subtraction in softmax: scores ~ N(0, 0.41^2), exp can't overflow.
"""

import ml_dtypes
import numpy as np

import concourse.bass as bass
import concourse.mybir as mybir
import concourse.tile as tile
from concourse import bacc
from concourse.bass_utils import run_bass_kernel_spmd
from concourse.masks import make_upper_triangular

P = 128
B = 2
L = 2048
E = 1024
H = 16
D = 64
HC = 4            # heads per core
F = HC * D        # 256: this core's slice of the head dim
EC = E // P       # 8 chunks of the embed dim
NLC = L // P      # 16 l-chunks
VST = NLC * 66    # v stride per head: 16 chunks of [64 v | 1 ones | 1 pad]

f32 = mybir.dt.float32
f32r = mybir.dt.float32r
bf16 = mybir.dt.bfloat16
AF = mybir.ActivationFunctionType
N_CORES = 8


def build_nc():
    nc = bacc.Bacc(None, target_bir_lowering=False, debug=False)

    xT = nc.dram_tensor("xT", [E, L], bf16, kind="ExternalInput")
    waT = nc.dram_tensor("waT", [E, 3 * F], bf16, kind="ExternalInput")
    woT = nc.dram_tensor("woT", [F, E], bf16, kind="ExternalInput")
    outT = nc.dram_tensor("outT", [E, L], bf16, kind="ExternalOutput")

    with tile.TileContext(nc) as tc:
        with (
            tc.tile_pool(name="persist", bufs=1) as pp,
            tc.tile_pool(name="qkv", bufs=1) as qp,
            tc.tile_pool(name="sps", bufs=2, space="PSUM") as sp,
            tc.tile_pool(name="ops", bufs=2, space="PSUM") as op_,
            tc.tile_pool(name="epool", bufs=3) as ep,
            tc.tile_pool(name="npool", bufs=4) as npl,
            tc.tile_pool(name="ob", bufs=3) as ob,
        ):
            # Persistent SBUF tensors.
            qT = [qp.tile([P, L], bf16, tag=f"q{p}", name=f"qT{p}") for p in range(2)]
            kT = [qp.tile([P, L], bf16, tag=f"k{p}", name=f"kT{p}") for p in range(2)]
            von = qp.tile([P, HC * VST], bf16, tag="von", name="von")
            oT = [qp.tile([P, L], bf16, tag=f"o{p}", name=f"oT{p}") for p in range(2)]
            wo_sb = [
                pp.tile([P, E], bf16, tag=f"wo{fc}", name=f"wo{fc}") for fc in range(2)
            ]
            x_sb = [
                qp.tile([P, L], bf16, tag=f"x{ec}", name=f"x{ec}") for ec in range(EC)
            ]
            wa_sb = [
                qp.tile([P, 3 * F], bf16, tag=f"wa{ec}", name=f"wa{ec}")
                for ec in range(EC)
            ]
            onesf = pp.tile([P, 64], f32, tag="onesf")
            trimask = pp.tile([P, P], bf16, tag="trimask")
            trimaskf = pp.tile([P, P], f32, tag="trimaskf")


            # Input DMAs, interleaved per e-chunk so the first QKV matmuls
            # (ec=0) can start as soon as possible; x l-half 1 and Wout after.
            for ec in range(EC):
                nc.sync.dma_start(wa_sb[ec][:], waT[ec * P : (ec + 1) * P, :])
                nc.sync.dma_start(
                    x_sb[ec][:, 0:1024], xT[ec * P : (ec + 1) * P, 0:1024]
                )
            for ec in range(EC):
                nc.sync.dma_start(
                    x_sb[ec][:, 1024:2048], xT[ec * P : (ec + 1) * P, 1024:2048]
                )
            for fc in range(2):
                nc.sync.dma_start(wo_sb[fc][:], woT[fc * P : (fc + 1) * P, :])

            # memset/affine_select can't encode bf16 targets: build f32, cast
            nc.gpsimd.memset(onesf[:], 1.0)
            # keep elements where j (partition) <= i (free): upper tri incl diag
            make_upper_triangular(nc, trimaskf[:], val=1.0, diag=True)
            nc.vector.tensor_copy(trimask[:], trimaskf[:])
            # ones/pad columns of von (Z rows): cols [64:66] of each 66-chunk
            for h in range(HC):
                dst = von[:].rearrange("p (g n t) -> p g n t", g=HC, t=66)[
                    :, h, :, 64:66
                ]
                nc.vector.tensor_copy(
                    dst, onesf[:, 0:32].rearrange("p (n t) -> p n t", t=2)
                )

            # ---------------- QKV unit emitters ----------------
            def emit_qk(kind, p, lb, eng):
                """kind 0 = q, 1 = k; produces (q|k)T[p][:, lb-half]."""
                ps = sp.tile([P, 1024], f32, tag="ps", name="ps_qk")
                off = kind * F + p * P
                for ec in range(EC):
                    for s in range(2):
                        nc.tensor.matmul(
                            ps[:, s * 512 : (s + 1) * 512],
                            wa_sb[ec][:, off : off + P],
                            x_sb[ec][
                                :, lb * 1024 + s * 512 : lb * 1024 + (s + 1) * 512
                            ],
                            start=(ec == 0),
                            stop=(ec == EC - 1),
                        )
                dst = (qT if kind == 0 else kT)[p][:, lb * 1024 : (lb + 1) * 1024]
                eng(dst, ps[:])

            def emit_v(lc, eng):
                """v natural [l, d] for all 4 heads at once (free dim 256)."""
                ps = sp.tile([P, F], f32, tag="ps", name="ps_v")
                for ec in range(EC):
                    nc.tensor.matmul(
                        ps[:],
                        x_sb[ec][:, lc * P : (lc + 1) * P],
                        wa_sb[ec][:, 2 * F : 3 * F],
                        start=(ec == 0),
                        stop=(ec == EC - 1),
                    )
                # scatter the 4 heads' [128, 64] into von (cast to bf16)
                dst = von[:].rearrange("p (g c) -> p g c", g=HC)[
                    :, :, lc * 66 : lc * 66 + 64
                ]
                src = ps[:].rearrange("p (g c) -> p g c", g=HC)
                eng(dst, src)

            def cp_scalar(dst, src):
                nc.scalar.copy(dst, src)

            def cp_vector(dst, src):
                nc.vector.tensor_copy(dst, src)

            # ---- serial head: everything attention sweep p0s0 needs ----
            emit_qk(0, 0, 0, cp_scalar)
            emit_qk(1, 0, 0, cp_vector)
            for lc in range(8):
                emit_v(lc, cp_scalar if lc % 2 == 0 else cp_vector)

            # ------- attention + dripped QKV/output-projection -------
            def emit_av(u):
                h, po, et, eoff, jc, a0, w, win0 = u
                for q in range((a0 - win0) // 512, (a0 - win0 + w - 1) // 512 + 1):
                    a = max(a0 - win0, q * 512)
                    bnd = min(a0 - win0 + w, (q + 1) * 512)
                    qg = win0 // 512 + q  # global 512-block of i
                    nc.tensor.matmul(
                        po[:, a:bnd],
                        von[:, h * VST + jc * 66 : h * VST + (jc + 1) * 66],
                        et[:, eoff + a - (a0 - win0) : eoff + bnd - (a0 - win0)],
                        start=(jc == 0),
                        stop=(jc == 4 * qg + 3),
                    )

            def emit_norm(p, win0, po2):
                # rows 0..63 of po are o^T, row 64 is Z. Bounce Z to SBUF
                # (the custom recip op can't read PSUM across banks), take
                # 1/Z on the DVE, broadcast it across the 64 o-partitions on
                # GPSIMD (no PE instruction -> no in-order PE stall).
                for hl in range(2):
                    zch = npl.tile([1, 1024], f32, tag="zch", name="zch")
                    nc.vector.tensor_copy(zch[:], po2[hl][64:65, :])
                    zfh = npl.tile([1, 1024], f32, tag="zfh", name="zfh")
                    nc.vector.reciprocal_approx_fast(zfh[:], zch[:])
                    zsB = npl.tile([64, 1024], f32, tag="zsB", name="zsB")
                    nc.gpsimd.partition_broadcast(zsB[:], zfh[:])
                    nc.vector.tensor_mul(
                        oT[p][hl * 64 : (hl + 1) * 64, win0 : win0 + 1024],
                        po2[hl][0:64, :],
                        zsB[:],
                    )

            def emit_oproj(oc, lb, eng):
                # output projection for e-chunk oc, l-half lb; both head
                # pairs accumulate in PSUM (fc = head pair = wo row chunk).
                ls = slice(lb * 1024, (lb + 1) * 1024)
                ps = sp.tile([P, 1024], f32, tag="ps", name="ps_op")
                for s in range(2):
                    for fc in range(2):
                        nc.tensor.matmul(
                            ps[:, s * 512 : (s + 1) * 512],
                            wo_sb[fc][:, oc * P : (oc + 1) * P],
                            oT[fc][
                                :, lb * 1024 + s * 512 : lb * 1024 + (s + 1) * 512
                            ],
                            start=(fc == 0),
                            stop=(fc == 1),
                        )
                ot = ob.tile([P, 1024], bf16, tag="ot", name="ot")
                eng(ot[:], ps[:])
                nc.sync.dma_start(outT[oc * P : (oc + 1) * P, ls], ot[:])

            # QKV work not needed by sweep p0s0, dripped into early sweeps.
            # q/k PSUM->SBUF copies ride the ScalarE (it has slack during
            # the sweep-0 streams); the strided v scatters stay on the DVE.
            filler = [
                lambda: emit_qk(0, 1, 0, cp_scalar),
                lambda: emit_qk(1, 1, 0, cp_scalar),
            ]
            filler += [lambda lc=lc: emit_v(8 + lc, cp_vector) for lc in range(8)]
            filler += [
                lambda: emit_qk(0, 0, 1, cp_scalar),
                lambda: emit_qk(1, 0, 1, cp_scalar),
                lambda: emit_qk(0, 1, 1, cp_scalar),
                lambda: emit_qk(1, 1, 1, cp_scalar),
            ]

            pending = []  # AV descriptors of the previous unit

            def mask_diag(et, off):
                nc.vector.tensor_mul(
                    et[:, off : off + P], et[:, off : off + P], trimask[:]
                )

            for sweep in range(2):  # i-window [1024*sweep, +1024)
                win0 = 1024 * sweep
                for p in range(2):  # head pair
                    po2 = [
                        op_.tile([66, 1024], f32, tag="po", name="po")
                        for _ in range(2)
                    ]
                    for jc in range((win0 + 1024) // P):
                        j0 = jc * P
                        a0 = max(j0, win0)
                        w = win0 + 1024 - a0
                        if w <= 512:
                            # narrow diagonal blocks: both heads in one PSUM
                            # tile -> a single wide exp. The heads' scores
                            # matmuls overlap on disjoint PE row groups, so
                            # they must land in different PSUM banks: h0 at
                            # col 0 (bank 0), h1 at col 512 (bank 1). The
                            # exp reads the two blocks with a strided AP and
                            # writes et packed [0:2w).
                            ps = sp.tile([P, 1024], f32, tag="ps", name="ps_s")
                            for hl in range(2):
                                hp = slice(hl * 64, (hl + 1) * 64)
                                nc.tensor.matmul(
                                    ps[:, hl * 512 : hl * 512 + w],
                                    kT[p][hp, j0 : j0 + P],
                                    qT[p][hp, a0 : a0 + w],
                                    start=True,
                                    stop=True,
                                )
                            et = ep.tile([P, 1024], bf16, tag="e", name="et")
                            nc.scalar.activation(
                                et[:, : 2 * w].rearrange(
                                    "p (b c) -> p b c", b=2
                                ),
                                ps[:].rearrange("p (b c) -> p b c", b=2)[
                                    :, :, 0:w
                                ],
                                AF.Exp,
                                scale=0.125,
                            )
                            mask_diag(et, 0)
                            mask_diag(et, w)
                            for u in pending:
                                emit_av(u)
                            pending = [
                                (2 * p, po2[0], et, 0, jc, a0, w, win0),
                                (2 * p + 1, po2[1], et, w, jc, a0, w, win0),
                            ]
                            if filler:
                                filler.pop(0)()
                            continue
                        for hl in range(2):
                            h = 2 * p + hl
                            hp = slice(hl * 64, (hl + 1) * 64)
                            ps = sp.tile([P, 1024], f32, tag="ps", name="ps_s")
                            for s0 in range(0, w, 512):
                                sw = min(512, w - s0)
                                nc.tensor.matmul(
                                    ps[:, s0 : s0 + sw],
                                    kT[p][hp, j0 : j0 + P],
                                    qT[p][hp, a0 + s0 : a0 + s0 + sw],
                                    start=True,
                                    stop=True,
                                )
                            et = ep.tile([P, 1024], bf16, tag="e", name="et")
                            nc.scalar.activation(
                                et[:, :w], ps[:, :w], AF.Exp, scale=0.125
                            )
                            if a0 == j0:
                                mask_diag(et, 0)
                            for u in pending:
                                emit_av(u)
                            pending = [(h, po2[hl], et, 0, jc, a0, w, win0)]
                            if filler:
                                filler.pop(0)()
                    filler.append(
                        lambda p=p, win0=win0, po2=po2: emit_norm(p, win0, po2)
                    )
                    if sweep == 0 and p == 1:
                        # both pairs' oT for l-half 0 complete after the
                        # norms above run; drip its output projection into
                        # the sweep-1 attention streams.
                        filler += [
                            lambda oc=oc: emit_oproj(oc, 0, cp_vector)
                            for oc in range(E // P)
                        ]
            for u in pending:
                emit_av(u)
            for f in filler:
                f()
            for oc in range(E // P):
                emit_oproj(oc, 1, cp_vector if oc % 2 == 0 else cp_scalar)

    nc.compile()
    return nc


def make_in_maps(x, Wa, Wout_w, Wout_b):
    """Host-side sharding: per-core input dicts."""
    x = np.asarray(x, dtype=np.float32)
    Wa = np.asarray(Wa, dtype=np.float32)
    Wout_w = np.asarray(Wout_w, dtype=np.float32)
    b16 = ml_dtypes.bfloat16

    xTs = [np.ascontiguousarray(x[b].T).astype(b16) for b in range(B)]
    in_maps = []
    for c in range(N_CORES):
        b, hg = divmod(c, 4)
        heads = list(range(4 * hg, 4 * hg + 4))
        qrows = np.concatenate([Wa[192 * h : 192 * h + 64] for h in heads], 0)
        krows = np.concatenate([Wa[192 * h + 64 : 192 * h + 128] for h in heads], 0)
        vrows = np.concatenate([Wa[192 * h + 128 : 192 * h + 192] for h in heads], 0)
        waT = np.ascontiguousarray(
            np.concatenate([qrows, krows, vrows], 0).T
        ).astype(b16)
        woT = np.ascontiguousarray(
            np.concatenate([Wout_w[:, 64 * h : 64 * h + 64] for h in heads], 1).T
        ).astype(b16)
        in_maps.append({"xT": xTs[b], "waT": waT, "woT": woT})
    return in_maps


def combine_outputs(core_outs, Wout_b):
    """core_outs: list of 8 outT [E, L] bf16 partials -> full [B, L, E]."""
    bias = np.asarray(Wout_b, np.float32)
    out = np.empty((B, L, E), np.float32)
    for b in range(B):
        acc = np.asarray(core_outs[4 * b], np.float32)
        for c in range(4 * b + 1, 4 * b + 4):
            acc = acc + np.asarray(core_outs[c], np.float32)
        out[b] = acc.T + bias
    return out


def kernel(x, Wa, Wout_w, Wout_b):
    nc = build_nc()
    in_maps = make_in_maps(x, Wa, Wout_w, Wout_b)
    res = run_bass_kernel_spmd(nc, in_maps, list(range(N_CORES)))
    return combine_outputs([r["outT"] for r in res.results], Wout_b)


if __name__ == "__main__":
    rng = np.random.default_rng(0)
    x = rng.standard_normal((B, L, E), dtype=np.float32)
    Wa = rng.standard_normal((3 * H * D, E), dtype=np.float32) * 0.02
    Ww = rng.standard_normal((E, H * D), dtype=np.float32) * 0.02
    Wb = rng.standard_normal((E,), dtype=np.float32) * 0.02
    out = kernel(x, Wa=Wa, Wout_w=Ww, Wout_b=Wb)
    print(out.shape, out.dtype)


# revision 25
# speedup vs baseline: 1.0716x; 1.0716x over previous
"""Multi-head causal attention (B=2, L=2048, E=1024, H=16, D=64) on 8 NeuronCores.

Sharding: data-parallel over batch x tensor-parallel over heads.
  core c: batch b = c // 4, head group hg = c % 4 -> heads [4*hg, 4*hg+4).
Each core computes QKV projection for its 4 heads, causal softmax attention,
and a *partial* output projection (its heads' slice of Wout). The host sums
the 4 bf16 partial outputs per batch in f32 and adds the bias.

Device schedule (single emission stream; Tile framework inserts sems):
  - inputs DMA'd chunked: waT, x l-half 0, x l-half 1, woT.
  - serial head: QKV for pair 0 / l-half 0 + V(all heads, l-half 0).
  - attention sweeps in order p0s0, p1s0, p0s1, p1s1 (i-windows of 1024);
    the remaining QKV units (pair 1 lh0, V lh1, all lh1) drip into the
    p0s0/p1s0 streams as fillers so the PE stays busy during exp waits.
  - output projection accumulates BOTH head pairs in PSUM (no HBM
    read-modify-write); lb0 units drip into the sweep-1 streams, lb1 is
    a short tail. Output is written bf16; bias is added on the host.
  - softmax Z: ones-column in V gives Z on PSUM row 64; 1/Z via DVE
    reciprocal, broadcast across 64 partitions by GPSIMD
    partition_broadcast (replaces the PE broadcast matmul + bounce).

Device notes:
  - Matmul operands bf16 (fp32 PSUM accumulation); host pre-casts/transposes.
  - Attention runs in the S^T layout (scores[j, i]).
  - No max-subtraction in softmax: scores ~ N(0, 0.41^2), exp can't overflow.
"""

import ml_dtypes
import numpy as np

import concourse.bass as bass
import concourse.mybir as mybir
import concourse.tile as tile
from concourse import bacc
from concourse.bass_utils import run_bass_kernel_spmd
from concourse.masks import make_upper_triangular

P = 128
B = 2
L = 2048
E = 1024
H = 16
D = 64
HC = 4            # heads per core
F = HC * D        # 256: this core's slice of the head dim
EC = E // P       # 8 chunks of the embed dim
NLC = L // P      # 16 l-chunks
VST = NLC * 66    # v stride per head: 16 chunks of [64 v | 1 ones | 1 pad]

f32 = mybir.dt.float32
f32r = mybir.dt.float32r
bf16 = mybir.dt.bfloat16
AF = mybir.ActivationFunctionType
N_CORES = 8


def build_nc():
    nc = bacc.Bacc(None, target_bir_lowering=False, debug=False)

    xT = nc.dram_tensor("xT", [E, L], bf16, kind="ExternalInput")
    waT = nc.dram_tensor("waT", [E, 3 * F], bf16, kind="ExternalInput")
    woT = nc.dram_tensor("woT", [F, E], bf16, kind="ExternalInput")
    outT = nc.dram_tensor("outT", [E, L], bf16, kind="ExternalOutput")

    with tile.TileContext(nc) as tc:
        with (
            tc.tile_pool(name="persist", bufs=1) as pp,
            tc.tile_pool(name="qkv", bufs=1) as qp,
            tc.tile_pool(name="sps", bufs=2, space="PSUM") as sp,
            tc.tile_pool(name="ops", bufs=2, space="PSUM") as op_,
            tc.tile_pool(name="epool", bufs=3) as ep,
            tc.tile_pool(name="npool", bufs=4) as npl,
            tc.tile_pool(name="ob", bufs=3) as ob,
        ):
            # Persistent SBUF tensors.
            qT = [qp.tile([P, L], bf16, tag=f"q{p}", name=f"qT{p}") for p in range(2)]
            kT = [qp.tile([P, L], bf16, tag=f"k{p}", name=f"kT{p}") for p in range(2)]
            von = qp.tile([P, HC * VST], bf16, tag="von", name="von")
            oT = [qp.tile([P, L], bf16, tag=f"o{p}", name=f"oT{p}") for p in range(2)]
            wo_sb = [
                pp.tile([P, E], bf16, tag=f"wo{fc}", name=f"wo{fc}") for fc in range(2)
            ]
            x_sb = [
                qp.tile([P, L], bf16, tag=f"x{ec}", name=f"x{ec}") for ec in range(EC)
            ]
            wa_sb = [
                qp.tile([P, 3 * F], bf16, tag=f"wa{ec}", name=f"wa{ec}")
                for ec in range(EC)
            ]
            onesf = pp.tile([P, 64], f32, tag="onesf")
            trimask = pp.tile([P, P], bf16, tag="trimask")
            trimaskf = pp.tile([P, P], f32, tag="trimaskf")


            # Input DMAs, interleaved per e-chunk so the first QKV matmuls
            # (ec=0) can start as soon as possible; x l-half 1 and Wout after.
            for ec in range(EC):
                nc.sync.dma_start(wa_sb[ec][:], waT[ec * P : (ec + 1) * P, :])
                nc.sync.dma_start(
                    x_sb[ec][:, 0:1024], xT[ec * P : (ec + 1) * P, 0:1024]
                )
            for ec in range(EC):
                nc.sync.dma_start(
                    x_sb[ec][:, 1024:2048], xT[ec * P : (ec + 1) * P, 1024:2048]
                )
            for fc in range(2):
                nc.sync.dma_start(wo_sb[fc][:], woT[fc * P : (fc + 1) * P, :])

            # memset/affine_select can't encode bf16 targets: build f32, cast
            nc.gpsimd.memset(onesf[:], 1.0)
            # keep elements where j (partition) <= i (free): upper tri incl diag
            make_upper_triangular(nc, trimaskf[:], val=1.0, diag=True)
            nc.vector.tensor_copy(trimask[:], trimaskf[:])
            # ones/pad columns of von (Z rows): cols [64:66] of each 66-chunk
            for h in range(HC):
                dst = von[:].rearrange("p (g n t) -> p g n t", g=HC, t=66)[
                    :, h, :, 64:66
                ]
                nc.vector.tensor_copy(
                    dst, onesf[:, 0:32].rearrange("p (n t) -> p n t", t=2)
                )

            # ---------------- QKV unit emitters ----------------
            def emit_qk(kind, p, lb, eng):
                """kind 0 = q, 1 = k; produces (q|k)T[p][:, lb-half]."""
                ps = sp.tile([P, 1024], f32, tag="ps", name="ps_qk")
                off = kind * F + p * P
                for ec in range(EC):
                    for s in range(2):
                        nc.tensor.matmul(
                            ps[:, s * 512 : (s + 1) * 512],
                            wa_sb[ec][:, off : off + P],
                            x_sb[ec][
                                :, lb * 1024 + s * 512 : lb * 1024 + (s + 1) * 512
                            ],
                            start=(ec == 0),
                            stop=(ec == EC - 1),
                        )
                dst = (qT if kind == 0 else kT)[p][:, lb * 1024 : (lb + 1) * 1024]
                eng(dst, ps[:])

            def emit_v(lc, eng):
                """v natural [l, d] for all 4 heads at once (free dim 256)."""
                ps = sp.tile([P, F], f32, tag="ps", name="ps_v")
                for ec in range(EC):
                    nc.tensor.matmul(
                        ps[:],
                        x_sb[ec][:, lc * P : (lc + 1) * P],
                        wa_sb[ec][:, 2 * F : 3 * F],
                        start=(ec == 0),
                        stop=(ec == EC - 1),
                    )
                # scatter the 4 heads' [128, 64] into von (cast to bf16)
                dst = von[:].rearrange("p (g c) -> p g c", g=HC)[
                    :, :, lc * 66 : lc * 66 + 64
                ]
                src = ps[:].rearrange("p (g c) -> p g c", g=HC)
                eng(dst, src)

            def cp_scalar(dst, src):
                nc.scalar.copy(dst, src)

            def cp_vector(dst, src):
                nc.vector.tensor_copy(dst, src)

            # ---- serial head: everything attention sweep p0s0 needs ----
            emit_qk(0, 0, 0, cp_scalar)
            emit_qk(1, 0, 0, cp_vector)
            for lc in range(8):
                emit_v(lc, cp_scalar if lc % 2 == 0 else cp_vector)

            # ------- attention + dripped QKV/output-projection -------
            def emit_av(u):
                h, po, et, eoff, jc, a0, w, win0 = u
                for q in range((a0 - win0) // 512, (a0 - win0 + w - 1) // 512 + 1):
                    a = max(a0 - win0, q * 512)
                    bnd = min(a0 - win0 + w, (q + 1) * 512)
                    qg = win0 // 512 + q  # global 512-block of i
                    nc.tensor.matmul(
                        po[:, a:bnd],
                        von[:, h * VST + jc * 66 : h * VST + (jc + 1) * 66],
                        et[:, eoff + a - (a0 - win0) : eoff + bnd - (a0 - win0)],
                        start=(jc == 0),
                        stop=(jc == 4 * qg + 3),
                    )

            def emit_norm(p, win0, po2):
                # rows 0..63 of po are o^T, row 64 is Z. Bounce Z to SBUF
                # (the custom recip op can't read PSUM across banks), take
                # 1/Z on the DVE, broadcast it across the 64 o-partitions on
                # GPSIMD (no PE instruction -> no in-order PE stall).
                for hl in range(2):
                    zch = npl.tile([1, 1024], f32, tag="zch", name="zch")
                    nc.vector.tensor_copy(zch[:], po2[hl][64:65, :])
                    zfh = npl.tile([1, 1024], f32, tag="zfh", name="zfh")
                    nc.vector.reciprocal_approx_fast(zfh[:], zch[:])
                    zsB = npl.tile([64, 1024], f32, tag="zsB", name="zsB")
                    nc.gpsimd.partition_broadcast(zsB[:], zfh[:])
                    nc.vector.tensor_mul(
                        oT[p][hl * 64 : (hl + 1) * 64, win0 : win0 + 1024],
                        po2[hl][0:64, :],
                        zsB[:],
                    )

            def emit_oproj(oc, lb, eng):
                # output projection for e-chunk oc, l-half lb; both head
                # pairs accumulate in PSUM (fc = head pair = wo row chunk).
                ls = slice(lb * 1024, (lb + 1) * 1024)
                ps = sp.tile([P, 1024], f32, tag="ps", name="ps_op")
                for s in range(2):
                    for fc in range(2):
                        nc.tensor.matmul(
                            ps[:, s * 512 : (s + 1) * 512],
                            wo_sb[fc][:, oc * P : (oc + 1) * P],
                            oT[fc][
                                :, lb * 1024 + s * 512 : lb * 1024 + (s + 1) * 512
                            ],
                            start=(fc == 0),
                            stop=(fc == 1),
                        )
                ot = ob.tile([P, 1024], bf16, tag="ot", name="ot")
                eng(ot[:], ps[:])
                nc.sync.dma_start(outT[oc * P : (oc + 1) * P, ls], ot[:])

            # QKV work not needed by sweep p0s0, dripped into early sweeps.
            # q/k PSUM->SBUF copies ride the ScalarE (it has slack during
            # the sweep-0 streams); the strided v scatters stay on the DVE.
            filler = [
                lambda: emit_qk(0, 1, 0, cp_vector),
                lambda: emit_qk(1, 1, 0, cp_vector),
            ]
            filler += [lambda lc=lc: emit_v(8 + lc, cp_vector) for lc in range(8)]
            filler += [
                lambda: emit_qk(0, 0, 1, cp_vector),
                lambda: emit_qk(1, 0, 1, cp_vector),
                lambda: emit_qk(0, 1, 1, cp_vector),
                lambda: emit_qk(1, 1, 1, cp_vector),
            ]

            pending = []  # AV descriptors of the previous unit

            def mask_diag(et, off):
                nc.vector.tensor_mul(
                    et[:, off : off + P], et[:, off : off + P], trimask[:]
                )

            for sweep in range(2):  # i-window [1024*sweep, +1024)
                win0 = 1024 * sweep
                for p in range(2):  # head pair
                    po2 = [
                        op_.tile([66, 1024], f32, tag="po", name="po")
                        for _ in range(2)
                    ]
                    for jc in range((win0 + 1024) // P):
                        j0 = jc * P
                        a0 = max(j0, win0)
                        w = win0 + 1024 - a0
                        if w <= 512:
                            # narrow diagonal blocks: both heads in one PSUM
                            # tile -> a single wide exp. The heads' scores
                            # matmuls overlap on disjoint PE row groups, so
                            # they must land in different PSUM banks: h0 at
                            # col 0 (bank 0), h1 at col 512 (bank 1). The
                            # exp reads the two blocks with a strided AP and
                            # writes et packed [0:2w).
                            ps = sp.tile([P, 1024], f32, tag="ps", name="ps_s")
                            for hl in range(2):
                                hp = slice(hl * 64, (hl + 1) * 64)
                                nc.tensor.matmul(
                                    ps[:, hl * 512 : hl * 512 + w],
                                    kT[p][hp, j0 : j0 + P],
                                    qT[p][hp, a0 : a0 + w],
                                    start=True,
                                    stop=True,
                                )
                            et = ep.tile([P, 1024], bf16, tag="e", name="et")
                            nc.scalar.activation(
                                et[:, : 2 * w].rearrange(
                                    "p (b c) -> p b c", b=2
                                ),
                                ps[:].rearrange("p (b c) -> p b c", b=2)[
                                    :, :, 0:w
                                ],
                                AF.Exp,
                                scale=0.125,
                            )
                            mask_diag(et, 0)
                            mask_diag(et, w)
                            for u in pending:
                                emit_av(u)
                            pending = [
                                (2 * p, po2[0], et, 0, jc, a0, w, win0),
                                (2 * p + 1, po2[1], et, w, jc, a0, w, win0),
                            ]
                            if filler:
                                filler.pop(0)()
                            continue
                        for hl in range(2):
                            h = 2 * p + hl
                            hp = slice(hl * 64, (hl + 1) * 64)
                            ps = sp.tile([P, 1024], f32, tag="ps", name="ps_s")
                            for s0 in range(0, w, 512):
                                sw = min(512, w - s0)
                                nc.tensor.matmul(
                                    ps[:, s0 : s0 + sw],
                                    kT[p][hp, j0 : j0 + P],
                                    qT[p][hp, a0 + s0 : a0 + s0 + sw],
                                    start=True,
                                    stop=True,
                                )
                            et = ep.tile([P, 1024], bf16, tag="e", name="et")
                            nc.scalar.activation(
                                et[:, :w], ps[:, :w], AF.Exp, scale=0.125
                            )
                            if a0 == j0:
                                mask_diag(et, 0)
                            for u in pending:
                                emit_av(u)
                            pending = [(h, po2[hl], et, 0, jc, a0, w, win0)]
                            if filler:
                                filler.pop(0)()
                    filler.append(
                        lambda p=p, win0=win0, po2=po2: emit_norm(p, win0, po2)
                    )
                    if sweep == 0 and p == 1:
                        # both pairs' oT for l-half 0 complete after the
                        # norms above run; drip its output projection into
                        # the sweep-1 attention streams.
                        filler += [
                            lambda oc=oc: emit_oproj(oc, 0, cp_vector)
                            for oc in range(E // P)
                        ]
            for u in pending:
                emit_av(u)
            for f in filler:
                f()
            for oc in range(E // P):
                emit_oproj(oc, 1, cp_vector if oc % 2 == 0 else cp_scalar)

    nc.compile()
    return nc


def make_in_maps(x, Wa, Wout_w, Wout_b):
    """Host-side sharding: per-core input dicts."""
    x = np.asarray(x, dtype=np.float32)
    Wa = np.asarray(Wa, dtype=np.float32)
    Wout_w = np.asarray(Wout_w, dtype=np.float32)
    b16 = ml_dtypes.bfloat16

    xTs = [np.ascontiguousarray(x[b].T).astype(b16) for b in range(B)]
    in_maps = []
    for c in range(N_CORES):
        b, hg = divmod(c, 4)
        heads = list(range(4 * hg, 4 * hg + 4))
        qrows = np.concatenate([Wa[192 * h : 192 * h + 64] for h in heads], 0)
        krows = np.concatenate([Wa[192 * h + 64 : 192 * h + 128] for h in heads], 0)
        vrows = np.concatenate([Wa[192 * h + 128 : 192 * h + 192] for h in heads], 0)
        waT = np.ascontiguousarray(
            np.concatenate([qrows, krows, vrows], 0).T
        ).astype(b16)
        woT = np.ascontiguousarray(
            np.concatenate([Wout_w[:, 64 * h : 64 * h + 64] for h in heads], 1).T
        ).astype(b16)
        in_maps.append({"xT": xTs[b], "waT": waT, "woT": woT})
    return in_maps


def combine_outputs(core_outs, Wout_b):
    """core_outs: list of 8 outT [E, L] bf16 partials -> full [B, L, E]."""
    bias = np.asarray(Wout_b, np.float32)
    out = np.empty((B, L, E), np.float32)
    for b in range(B):
        acc = np.asarray(core_outs[4 * b], np.float32)
        for c in range(4 * b + 1, 4 * b + 4):
            acc = acc + np.asarray(core_outs[c], np.float32)
        out[b] = acc.T + bias
    return out


def kernel(x, Wa, Wout_w, Wout_b):
    nc = build_nc()
    in_maps = make_in_maps(x, Wa, Wout_w, Wout_b)
    res = run_bass_kernel_spmd(nc, in_maps, list(range(N_CORES)))
    return combine_outputs([r["outT"] for r in res.results], Wout_b)


if __name__ == "__main__":
    rng = np.random.default_rng(0)
    x = rng.standard_normal((B, L, E), dtype=np.float32)
    Wa = rng.standard_normal((3 * H * D, E), dtype=np.float32) * 0.02
    Ww = rng.standard_normal((E, H * D), dtype=np.float32) * 0.02
    Wb = rng.standard_normal((E,), dtype=np.float32) * 0.02
    out = kernel(x, Wa=Wa, Wout_w=Ww, Wout_b=Wb)
    print(out.shape, out.dtype)
